# revision 73
# baseline (speedup 1.0000x reference)
"""Trainium2 Bass kernel for nn_BetaModel_5660766896152 (7-layer dense
transformer, D=280, H=7, T=512, B=32, V=256, tied embeddings, RoPE, SwiGLU).

Strategy: data-parallel over batch — 8 cores x 4 sequences, weights
replicated, no collectives. Each core runs the full model on its 4
sequences; the host shards inputs and concatenates outputs.

v2 restructure (from trace analysis of the v1 kernel):
 - Softmax denominators: 1/x via exp(-ln(x)) on ACT (same table set as the
   attention exp), replacing 3.3us single-partition DVE RECIPROCALs that
   stalled the PE ~320us/run.
 - RMSNorm: mean-square is broadcast to all 128 partitions inside the ones
   matmul itself (lhsT = ones [128,128]), so Ln/Exp produce the broadcast
   1/rms directly — no second broadcast matmul, no copy.
 - RoPE: rotate_half(q) computed from q by a shared 128x128 permutation
   matmul (sign folded into the sin table) instead of a second full
   projection per tensor — 16 fewer N=512 matmuls per seq-layer.
 - Attention per head-pair (2 heads/PSUM bank at rows 0/64): per s-block,
   score+mask MMs for both heads -> one exp ACT -> col-paired PV MMs; the
   next s-block's score MMs run while the previous block's exp is on ACT.
 - Fixed 8-bank PSUM plan with only top-level pools: ps2 = 2x[128,2,T]
   rotating (qkv proj halves / rotate-half / score pairs / mlp gate+up),
   po = 3x[128,T] (embed gather, o pairs + denom bcast, WO/W2 accums),
   pms = 1x[128,T] (norm mean-square / logits).
 - norm2 at each seq's attention tail (same ACT table set as exp); norm1
   batched after the MLP phase to keep Silu<->Ln/Exp table switches at 2
   per layer.

v3 restructure (trace-driven, from 2.13ms to 1.88ms device exec):
 - Deferred attention tails: each pair's PSUM accumulator is copied to
   SBUF as soon as PV finishes (bank frees without waiting on the
   denominator); the denominator/WO/x+=/norm2 tail of seq s is emitted
   after seq s+1's projection matmuls so its matmuls never stall the PE.
 - Denominator rows of all 7 heads DMA-gather (SBUF->SBUF) into one
   packed tile: one Ln + one Exp per sequence instead of 14 single-row
   ACT ops (-230us ACT); per-pair bf16 broadcast matmuls (1 cyc/row).
 - W2 matmuls lag one fc iteration behind gate/up so the silu->mult
   chain is always satisfied when the PE reaches them.
 - Seq-3's tail splits: ACT part right after its pairs, matmul part
   after MLP(s0); next layer's norm1(s0) rides the same lnexp table
   window, so each attention phase starts with h1(s0) ready.
 - QK copies split q->DVE / k->ACT; perm+rope interleaved per chunk
   half; V projection moved after perm (single po banks) to cover the
   rope chain; final logits lag one seq behind the final norms.

v4 (1.88 -> ~1.83ms device): the whole s3 tail (Ln/Exp included) rides
the lnexp window after the first two MLP seqs, so the silu table load
starts the moment pairs end; MLP order (s1, s0, tail, norm1(s0'),
norm1(s1'), s2, s3) precomputes two sequences' h1 per layer with no
extra table loads; norm2's square runs on GpSimd (slack-rich, off the
ACT pacer). Attention is ACT-throughput-bound (~24us/seq exp+copies vs
~21us PE), so remaining headroom needs exp/copy reduction, not more PE
overlap.
"""

import numpy as np

# ---------------------------------------------------------------- constants
B, T, D, H, HD, L, FF, V = 32, 512, 280, 7, 40, 7, 1120, 256
ROT = HD // 2  # 20
DP = 384  # padded D, 3 chunks
NDC = 3
EP = 512  # padded head-feature dim, 4 chunks, head h at chunk h//2 offset 64*(h%2)
NEC = 4
FFP = 1152  # padded FF, 9 chunks
NFC = 9
NVC = 2  # V chunks
NSEQ = 4  # sequences per core
NCORES = 8
NTC = 4  # t chunks of 128
NPAIR = 4  # head pairs (pair 3 = head 6 alone)
SCALE = float(HD) ** -0.5
MASKV = -200.0  # per (s-t) step added pre-scale to masked scores
EPS = 1e-6

_CACHE = {}


def _e_idx(h, r):
    return 128 * (h // 2) + 64 * (h % 2) + r


def _v_off(r):
    # within a 64-col v/o block: col 0 = softmax-denominator ones, dims at 1-40
    return r + 1


def _bf16(a):
    import ml_dtypes

    return np.asarray(a, dtype=ml_dtypes.bfloat16)


def _prep_weights(inputs):
    """Host-side weight prep shared by all cores. Returns dict name->np array."""
    f32 = lambda a: np.asarray(a, dtype=np.float32)
    embed = f32(inputs["embed_w"])  # [V, D]
    wq, wk, wv, wo = (f32(inputs[k]) for k in ("wq", "wk", "wv", "wo"))
    w1, w2, w3 = (f32(inputs[k]) for k in ("w1", "w2", "w3"))
    n1, n2, nw = f32(inputs["n1_w"]), f32(inputs["n2_w"]), f32(inputs["norm_w"])

    def qk_lhsT(w, n1w):  # [D_out=280, D_in=280] -> [NDC, 128, EP] lhsT (bf16)
        we = w * n1w[None, :]  # fold norm weight on input dim
        big = np.zeros((DP, EP), np.float32)  # [d, e']
        for h in range(H):
            for r_ in range(HD):
                big[:D, _e_idx(h, r_)] = we[h * HD + r_, :]
        return _bf16(big.reshape(NDC, 128, EP))

    def wv_rhs(w, n1w):
        # [NDC, 128, 448] rhs; 64-stride head cols; within a block col 0 is the
        # softmax-denominator ones column, dims at 1..40, rest zero.
        we = w * n1w[None, :]
        big = np.zeros((DP, 7 * 64), np.float32)
        for h in range(H):
            for r_ in range(HD):
                big[:D, 64 * h + _v_off(r_)] = we[h * HD + r_, :]
        return _bf16(big.reshape(NDC, 128, 7 * 64))

    def wo_lhsT(w):
        # [NEC, 128, DP] lhsT over the o layout: head h -> chunk h//2, row
        # offset 64*(h%2) + _v_off(r) (matching wv_rhs's within-block layout)
        big = np.zeros((EP, DP), np.float32)
        for h in range(H):
            for r_ in range(HD):
                big[128 * (h // 2) + 64 * (h % 2) + _v_off(r_), :D] = w[:, h * HD + r_]
        return _bf16(big.reshape(NEC, 128, DP))

    def w13_lhsT(w, n2w):  # [FF, D] -> [NDC, 128, FFP]
        we = w * n2w[None, :]
        big = np.zeros((DP, FFP), np.float32)
        big[:D, :FF] = we.T
        return _bf16(big.reshape(NDC, 128, FFP))

    def w2_lhsT(w):  # [D, FF] -> [NFC, 128, DP]
        big = np.zeros((FFP, DP), np.float32)
        big[:FF, :D] = w.T
        return _bf16(big.reshape(NFC, 128, DP))

    c = {}
    c["wq"] = np.stack([qk_lhsT(wq[l], n1[l]) for l in range(L)])
    c["wk"] = np.stack([qk_lhsT(wk[l], n1[l]) for l in range(L)])
    c["wv"] = np.stack([wv_rhs(wv[l], n1[l]) for l in range(L)])
    c["wo"] = np.stack([wo_lhsT(wo[l]) for l in range(L)])
    c["w1"] = np.stack([w13_lhsT(w1[l], n2[l]) for l in range(L)])
    c["w3"] = np.stack([w13_lhsT(w3[l], n2[l]) for l in range(L)])
    c["w2"] = np.stack([w2_lhsT(w2[l]) for l in range(L)])

    # combined 3rd-D-chunk (24 contraction rows) weight tiles: partner rows
    # at partitions 32/64/96 so up to four matmuls run concurrently in the
    # PE array via 32-row tile groups.
    def packn(blocks):
        m = np.zeros((128,) + blocks[0].shape[1:], np.float32)
        for i, b in enumerate(blocks):
            m[32 * i : 32 * i + 24] = np.asarray(b[0:24], np.float32)
        return _bf16(m)

    c["wqk2"] = np.stack(
        [packn([c["wq"][l][2], c["wk"][l][2]] * 2) for l in range(L)]
    )
    c["w13c2"] = np.stack(
        [packn([c["w1"][l][2], c["w3"][l][2]] * 2) for l in range(L)]
    )
    c["wv2p"] = np.stack([packn([c["wv"][l][2]] * 2) for l in range(L)])

    emb_pad = np.zeros((V, DP), np.float32)
    emb_pad[:, :D] = embed
    c["emb"] = emb_pad.reshape(NVC, 128, DP)  # fp32 lhsT for exact gather
    embT = np.zeros((DP, V), np.float32)
    embT[:D, :] = (embed * nw[None, :]).T
    c["embT"] = _bf16(embT.reshape(NDC, 128, V))

    inv = 1.0 / (10000.0 ** (np.arange(0, HD, 2, dtype=np.float32) / HD))
    tt = np.arange(T, dtype=np.float32)
    fr = tt[:, None] * inv[None, :]  # [T, ROT]
    cos = np.cos(np.concatenate([fr, fr], -1))  # [T, HD]
    sin = np.sin(np.concatenate([fr, fr], -1))
    cosf = np.zeros((EP, T), np.float32)
    sinf = np.zeros((EP, T), np.float32)
    for h in range(H):
        for r_ in range(HD):
            cosf[_e_idx(h, r_)] = cos[:, r_]
            # rotate_half sign folded into sin: out row r<ROT uses -q[r+ROT],
            # row r>=ROT uses +q[r-ROT]; the perm matrix below is unsigned.
            sgn = -1.0 if r_ < ROT else 1.0
            sinf[_e_idx(h, r_)] = sgn * sin[:, r_]
    c["cos"] = _bf16(cosf.reshape(NEC, 128, T))
    c["sin"] = _bf16(sinf.reshape(NEC, 128, T))

    m = np.arange(128)
    lt = (m[:, None] <= m[None, :]).astype(np.float32) * MASKV  # [m, s]
    rt = (m[:, None] >= m[None, :] + 1).astype(np.float32)  # [m, t]
    c["lt"] = _bf16(lt)
    c["rt"] = _bf16(rt)

    # unsigned rotate-half permutation (within each 64-row block)
    perm = np.zeros((128, 128), np.float32)
    for b in (0, 64):
        for r_ in range(ROT):
            perm[b + r_ + ROT, b + r_] = 1.0  # out[r] = q[r+ROT]
            perm[b + r_, b + r_ + ROT] = 1.0  # out[r+ROT] = q[r]
    c["perm"] = _bf16(perm)

    # per-pair broadcast of the packed reciprocal-denominator rows: pair p's
    # matmul maps rd8 row 2p -> out rows 0..63 and row 2p+1 -> rows 64..127
    bc = np.zeros((NPAIR, 128, 128), np.float32)
    for p in range(NPAIR):
        bc[p, 2 * p, 0:64] = 1.0
        if p < NPAIR - 1:
            bc[p, 2 * p + 1, 64:128] = 1.0
    c["bc"] = _bf16(bc)

    c["onerb"] = _bf16(np.ones((128, 128), np.float32))
    return c


def _prep_onehot(idx_core):  # [n, T] -> [n, 128, NVC, T] fp32
    n = idx_core.shape[0]
    oh = np.zeros((n, 128, NVC, T), np.float32)
    for s in range(n):
        for vc in range(NVC):
            sel = (idx_core[s][None, :] == (vc * 128 + np.arange(128))[:, None])
            oh[s, :, vc, :] = sel.astype(np.float32)
    return oh


# ---------------------------------------------------------------- bass build
def _build(n_seqs=NSEQ, n_layers=L):
    import concourse.bass as bass
    import concourse.mybir as mybir
    import concourse.tile as tile_mod

    _patch_tail_drain(tile_mod)

    dt = mybir.dt
    F = mybir.ActivationFunctionType
    OP = mybir.AluOpType

    nc = bass.Bass("TRN2", debug=False, num_devices=NCORES)

    def din(name, shape, dty=dt.bfloat16):
        return nc.dram_tensor(name, shape, dty, kind="ExternalInput")

    d = {}
    d["oh"] = din("oh", [n_seqs, 128, NVC, T], dt.float32r)
    d["emb"] = din("emb", [NVC, 128, DP], dt.float32r)
    d["embT"] = din("embT", [NDC, 128, V])
    d["cos"] = din("cos", [NEC, 128, T])
    d["sin"] = din("sin", [NEC, 128, T])
    d["lt"] = din("lt", [128, 128])
    d["rt"] = din("rt", [128, 128])
    d["perm"] = din("perm", [128, 128])
    d["bc"] = din("bc", [NPAIR, 128, 128])
    d["onerb"] = din("onerb", [128, 128])
    d["wq"] = din("wq", [n_layers, NDC, 128, EP])
    d["wk"] = din("wk", [n_layers, NDC, 128, EP])
    d["wv"] = din("wv", [n_layers, NDC, 128, 7 * 64])
    d["wo"] = din("wo", [n_layers, NEC, 128, DP])
    d["w1"] = din("w1", [n_layers, NDC, 128, FFP])
    d["w3"] = din("w3", [n_layers, NDC, 128, FFP])
    d["w2"] = din("w2", [n_layers, NFC, 128, DP])
    d["wqk2"] = din("wqk2", [n_layers, 128, EP])
    d["w13c2"] = din("w13c2", [n_layers, 128, FFP])
    d["wv2p"] = din("wv2p", [n_layers, 128, 7 * 64])
    logits = nc.dram_tensor("logits", [n_seqs, NTC, 128, V], dt.float32, kind="ExternalOutput")

    MM = nc.tensor.matmul
    ACT = nc.scalar.activation
    TT = nc.vector.tensor_tensor

    with tile_mod.TileContext(nc) as tc:
        with (
            nc.allow_low_precision(reason="f32r broadcasts/gather keep >=19-bit mantissa"),
            tc.tile_pool(name="consts", bufs=1) as cpool,
            tc.tile_pool(name="weights", bufs=2) as wpool,
            tc.tile_pool(name="xres", bufs=1) as xpool,
            tc.tile_pool(name="hres", bufs=1) as hpool,
            tc.tile_pool(name="b1", bufs=1) as b1pool,
            tc.tile_pool(name="acts", bufs=2) as apool,
            tc.tile_pool(name="small", bufs=2) as spool,
            tc.tile_pool(name="ps2", bufs=2, space="PSUM") as ps2,   # 2x[128,2,T] = 4 banks
            tc.tile_pool(name="po", bufs=3, space="PSUM") as po,     # 3x[128,T]   = 3 banks
            tc.tile_pool(name="pms", bufs=1, space="PSUM") as pms,   # 1x[128,T]   = 1 bank
        ):
            # ---- constants resident in SBUF
            cos_sb = cpool.tile([128, NEC, T], dt.bfloat16, name="cos_sb")
            sin_sb = cpool.tile([128, NEC, T], dt.bfloat16, name="sin_sb")
            for c in range(NEC):
                nc.sync.dma_start(cos_sb[:, c], d["cos"].ap()[c])
                nc.sync.dma_start(sin_sb[:, c], d["sin"].ap()[c])
            lt_sb = cpool.tile([128, 128], dt.bfloat16, name="lt_sb")
            rt_sb = cpool.tile([128, 128], dt.bfloat16, name="rt_sb")
            perm_sb = cpool.tile([128, 128], dt.bfloat16, name="perm_sb")
            onerb_sb = cpool.tile([128, 128], dt.bfloat16, name="onerb_sb")
            for t_, n_ in ((lt_sb, "lt"), (rt_sb, "rt"), (perm_sb, "perm"),
                           (onerb_sb, "onerb")):
                nc.sync.dma_start(t_[:], d[n_].ap())
            bc_sb = cpool.tile([128, NPAIR, 128], dt.bfloat16, name="bc_sb")
            for p in range(NPAIR):
                nc.sync.dma_start(bc_sb[:, p], d["bc"].ap()[p])
            emb_t = []
            for vc in range(NVC):
                et = cpool.tile([128, DP], dt.float32r, name=f"emb{vc}", tag=f"emb{vc}")
                nc.sync.dma_start(et[:], d["emb"].ap()[vc])
                emb_t.append(et)
            embT_t = []
            for kc in range(NDC):
                et = cpool.tile([128, V], dt.bfloat16, name=f"embT{kc}", tag=f"embT{kc}")
                nc.sync.dma_start(et[:], d["embT"].ap()[kc])
                embT_t.append(et)
            # packed denominator rows (head h at row h, DMA-gathered from the
            # pair accumulators) -> one Ln + one Exp per sequence; rd8's junk
            # rows are memset once for the finite-checker / broadcast matmuls.
            den8 = cpool.tile([128, T], dt.bfloat16, name="den8")
            ld8 = cpool.tile([128, T], dt.float32, name="ld8")
            rd8 = cpool.tile([128, T], dt.bfloat16, name="rd8")
            nc.any.memset(rd8[:], 0.0)
            eps_t = cpool.tile([128, 1], dt.float32, name="eps_t")
            nc.any.memset(eps_t[:], EPS)

            from concourse.tile import add_dep_helper

            # ACT table-set phase ordering: Silu lives in a different table
            # set than Ln/Exp; keep set switches to 2/layer by pinning the
            # first op of each set-phase after the last op of the previous.
            actdep = {"last_lnexp": None, "last_silu": None}

            def _dep_after(inst, prev):
                if prev is not None:
                    add_dep_helper(inst.ins, prev.ins, sync=False,
                                   reason="act-table-set phase order")

            def norm_h(x, tag, dst_tag, after_silu=False, sq_gpsimd=False):
                # x [128, NDC, T] fp32 sbuf -> h bf16 [128, NDC, T].
                # 1/rms lands broadcast on all 128 partitions via ones-lhsT.
                # sq_gpsimd: slack-rich norms square on GpSimd to relieve the
                # ACT pacer (GpSimd is ~2.5x slower but otherwise idle).
                x2 = b1pool.tile([128, NDC, T], dt.bfloat16, name=f"x2{tag}", tag="x2")
                if sq_gpsimd:
                    nc.gpsimd.tensor_tensor(x2[:], x[:], x[:], OP.mult)
                else:
                    ACT(x2[:], x[:], F.Square)
                ms = pms.tile([128, T], dt.float32, name=f"ms{tag}", tag="ms")
                for kc in range(NDC):
                    MM(ms[:], onerb_sb[:], x2[:, kc], start=(kc == 0), stop=(kc == NDC - 1))
                lms = b1pool.tile([128, T], dt.float32, name=f"lms{tag}", tag="lms")
                li = ACT(lms[:], ms[:], F.Ln, scale=1.0 / D, bias=eps_t[:])
                if after_silu:
                    _dep_after(li, actdep["last_silu"])
                rbc = b1pool.tile([128, T], dt.float32, name=f"rbc{tag}", tag="rbc")
                ei = ACT(rbc[:], lms[:], F.Exp, scale=-0.5)
                actdep["last_lnexp"] = ei
                h_ = hpool.tile([128, NDC, T], dt.bfloat16, name=f"h{tag}", tag=dst_tag)
                # chunk 2 first: its replica DMA (rows 0..23 -> 32..55, which
                # lets paired 3rd-chunk matmuls run as concurrent 32-row PE
                # tiles) issues early so the queue latency hides under the
                # remaining mults and the kc01 projection matmuls
                TT(h_[:, 2], x[:, 2], rbc[:], OP.mult)
                for ro in (32, 64, 96):
                    nc.sync.dma_start(h_[ro : ro + 24, 2, :], h_[0:24, 2, :])
                for kc in range(2):
                    TT(h_[:, kc], x[:, kc], rbc[:], OP.mult)
                return h_

            # ---- embedding for all seqs -> x (fp32, exact gather); norm1
            # happens at the top of each layer's attention phase so its chain
            # hides under the previous sequence's attention matmuls.
            xs = [None] * n_seqs
            h2 = [None] * n_seqs
            h1cache = {}
            for s in range(n_seqs):
                oh_sb = b1pool.tile([128, NVC, T], dt.float32r, name=f"oh{s}", tag="oh")
                nc.sync.dma_start(oh_sb[:], d["oh"].ap()[s])
                xt = xpool.tile([128, NDC, T], dt.float32, name=f"x{s}", tag=f"x{s}")
                for mc in range(NDC):
                    xe = po.tile([128, T], dt.float32, name=f"xe{s}_{mc}", tag="po")
                    for vc in range(NVC):
                        MM(
                            xe[:],
                            emb_t[vc][:, 128 * mc : 128 * mc + 128],
                            oh_sb[:, vc],
                            start=(vc == 0),
                            stop=(vc == NVC - 1),
                        )
                    ACT(xt[:, mc], xe[:], F.Copy)
                xs[s] = xt

            for l in range(n_layers):
                wt = {}
                for wname, nchunk in (
                    ("wq", 2), ("wk", 2), ("wv", 2), ("wo", NEC),
                    ("w1", 2), ("w3", 2), ("w2", NFC),
                ):
                    tiles = []
                    for kc in range(nchunk):
                        wtile = wpool.tile(
                            [128, d[wname].shape[-1]], dt.bfloat16,
                            name=f"{wname}_{kc}", tag=f"{wname}_{kc}",
                        )
                        nc.sync.dma_start(wtile[:], d[wname].ap()[l, kc])
                        tiles.append(wtile)
                    wt[wname] = tiles
                for wname in ("wqk2", "w13c2", "wv2p"):
                    wtile = wpool.tile(
                        [128, d[wname].shape[-1]], dt.bfloat16,
                        name=wname, tag=wname,
                    )
                    nc.sync.dma_start(wtile[:], d[wname].ap()[l])
                    wt[wname] = wtile

                # ======== attention phase (lnexp ACT table) ========
                # per seq: norm1 -> QK proj -> perm/rope -> [deferred tail of
                # the previous seq] -> V proj -> score/PV pairs. Each pair's
                # accumulator is copied off PSUM as soon as PV finishes (bank
                # frees without waiting on the denominator); the denominator /
                # WO / x+= / norm2 tail is deferred so its matmuls sit behind
                # the next sequence's projection matmuls and never stall.
                def tail_act(s, o_raw, o_sb):
                    # all 7 heads' reciprocal denominators in one Ln + one Exp
                    ACT(ld8[0:7, :], den8[0:7, :], F.Ln)
                    ei = ACT(rd8[0:7, :], ld8[0:7, :], F.Exp, scale=-1.0)
                    actdep["last_lnexp"] = ei

                def tail_mm(s, o_raw, o_sb):
                    for p in range(NPAIR):
                        nh = 1 if p == 3 else 2
                        nrow = 64 * nh
                        rbp = pms.tile([128, T], dt.float32, name=f"rbp{p}", tag="ms")
                        MM(rbp[0:nrow, :], bc_sb[:, p, 0:nrow], rd8[:, :], start=True, stop=True)
                        TT(o_sb[0:nrow, p], o_raw[p][0:nrow, :], rbp[0:nrow, :], OP.mult)
                    xacc = [po.tile([128, T], dt.float32, name=f"xac{mc}", tag="po")
                            for mc in range(NDC)]
                    for kc in range(NEC):
                        for mc in range(NDC):
                            MM(
                                xacc[mc][:],
                                wt["wo"][kc][:, 128 * mc : 128 * mc + 128],
                                o_sb[:, kc],
                                start=(kc == 0),
                                stop=(kc == NEC - 1),
                            )
                    for mc in range(NDC):
                        TT(xs[s][:, mc], xacc[mc][:], xs[s][:, mc], OP.add)
                    h2[s] = norm_h(xs[s], f"n2_{s}_{l}", f"h2_{s}", sq_gpsimd=True)

                def attn_tail(s, o_raw, o_sb):
                    tail_act(s, o_raw, o_sb)
                    tail_mm(s, o_raw, o_sb)

                pend = None
                first_norm = True
                for s in range(n_seqs):
                    h1s = h1cache.pop(s, None)
                    if h1s is None:
                        h1s = norm_h(xs[s], f"n1_{s}_{l}", f"h1_{s}",
                                     after_silu=(l > 0 and first_norm))
                        first_norm = False

                    # ---- Q/K projections; per-chunk PSUM->SBUF copies so the
                    # perm matmuls can start as soon as a chunk lands
                    qrot = apool.tile([128, NEC, T], dt.bfloat16, name="qrot", tag="qrot")
                    krot = apool.tile([128, NEC, T], dt.bfloat16, name="krot", tag="krot")
                    q_sb = b1pool.tile([128, NEC, T], dt.bfloat16, name="q_sb", tag="q_sb")
                    k_sb = b1pool.tile([128, NEC, T], dt.bfloat16, name="k_sb", tag="k_sb")
                    for hb in range(2):  # chunk halves {0,1}, {2,3}
                        c0 = 2 * hb
                        pps = {}
                        for wname in ("wq", "wk"):
                            pp = ps2.tile([128, 2, T], dt.float32, name=f"p{wname}{hb}", tag="p2")
                            for ci in range(2):
                                for kc in range(2):
                                    MM(
                                        pp[:, ci],
                                        wt[wname][kc][:, 128 * (c0 + ci) : 128 * (c0 + ci) + 128],
                                        h1s[:, kc],
                                        start=(kc == 0),
                                        stop=False,
                                    )
                            pps[wname] = pp
                        # 3rd-chunk (24-row) q/k matmuls for both column
                        # chunks run as FOUR concurrent 32-row PE tiles
                        for ci in range(2):
                            cw = 128 * (c0 + ci)
                            rq, rk = 64 * ci, 64 * ci + 32
                            MM(
                                pps["wq"][:, ci],
                                wt["wqk2"][rq : rq + 24, cw : cw + 128],
                                h1s[rq : rq + 24, 2],
                                start=False,
                                stop=True,
                                skip_group_check=True,
                                tile_position=(rq, 0),
                            )
                            MM(
                                pps["wk"][:, ci],
                                wt["wqk2"][rk : rk + 24, cw : cw + 128],
                                h1s[rk : rk + 24, 2],
                                start=False,
                                stop=True,
                                skip_group_check=True,
                                tile_position=(rk, 0),
                            )
                        # q copies on DVE, k copies on ACT: halves the copy
                        # backlog ahead of the rope chain
                        for ci in range(2):
                            nc.vector.tensor_copy(q_sb[:, c0 + ci], pps["wq"][:, ci])
                            ACT(k_sb[:, c0 + ci], pps["wk"][:, ci], F.Copy)
                        for ci in range(2):
                            cch = c0 + ci
                            p2 = ps2.tile([128, 2, T], dt.float32, name=f"p2_{cch}", tag="p2")
                            MM(p2[:, 0], perm_sb[:], q_sb[:, cch], start=True, stop=True)
                            MM(p2[:, 1], perm_sb[:], k_sb[:, cch], start=True, stop=True)
                            t1 = spool.tile([128, T], dt.bfloat16, name="t1", tag="t1")
                            t2 = spool.tile([128, T], dt.bfloat16, name="t2", tag="t2")
                            TT(t1[:], q_sb[:, cch], cos_sb[:, cch], OP.mult)
                            TT(t2[:], p2[:, 0], sin_sb[:, cch], OP.mult)
                            TT(qrot[:, cch], t1[:], t2[:], OP.add)
                            t3 = spool.tile([128, T], dt.bfloat16, name="t3", tag="t1")
                            t4 = spool.tile([128, T], dt.bfloat16, name="t4", tag="t2")
                            TT(t3[:], k_sb[:, cch], cos_sb[:, cch], OP.mult)
                            TT(t4[:], p2[:, 1], sin_sb[:, cch], OP.mult)
                            TT(krot[:, cch], t3[:], t4[:], OP.add)

                    # ---- previous seq's deferred tail: its matmuls queue
                    # behind this seq's projections, chains long satisfied
                    if pend is not None:
                        attn_tail(*pend)
                        pend = None

                    # ---- V (token-major, with denominator ones columns),
                    # one po bank per t-chunk
                    v_sb = apool.tile([128, NTC, 448], dt.bfloat16, name="v_sb", tag="v_sb")
                    for vb in range(2):  # token-chunk pairs; kc2 tiles packed
                        vps = []
                        for ti in range(2):
                            tc_ = 2 * vb + ti
                            vp = po.tile([128, T], dt.float32, name=f"vp{tc_}", tag="po")
                            for kc in range(2):
                                MM(
                                    vp[:, 0:448],
                                    h1s[:, kc, 128 * tc_ : 128 * tc_ + 128],
                                    wt["wv"][kc][:],
                                    start=(kc == 0),
                                    stop=False,
                                )
                            vps.append(vp)
                        for ti in range(2):
                            tc_ = 2 * vb + ti
                            ro = 32 * ti
                            MM(
                                vps[ti][:, 0:448],
                                h1s[ro : ro + 24, 2, 128 * tc_ : 128 * tc_ + 128],
                                wt["wv2p"][ro : ro + 24, :],
                                start=False,
                                stop=True,
                                skip_group_check=True,
                                tile_position=(ro, 0),
                            )
                        for ti in range(2):
                            tc_ = 2 * vb + ti
                            ACT(v_sb[:, tc_], vps[ti][:, 0:448], F.Copy)
                            nc.any.memset(v_sb[:, tc_, 0 : 1 + 6 * 64 : 64], 1.0)

                    # ---- attention, two head-pairs in flight (heads 2p, 2p+1
                    # at rows 0/64 of a pair's PSUM bank); interleaving pairs
                    # keeps score MMs streaming while the other pair's exp runs.
                    o_sb = apool.tile([128, NEC, T], dt.bfloat16, name="o_sb", tag="o_sb")
                    nc.any.memset(o_sb[64:128, 3], 0.0)
                    o_raw = {}
                    for pg in ((0, 1), (2, 3)):
                        E_t, o_t = {}, {}
                        for p in pg:
                            E_t[p] = apool.tile([128, 2, T], dt.bfloat16,
                                                name=f"E{p}", tag=f"E{p % 2}")
                            o_t[p] = po.tile([128, T], dt.float32, name=f"o_{p}", tag="po")
                        for cc in range(NTC):
                            for p in pg:
                                nh = 1 if p == 3 else 2
                                E_sb, o_p = E_t[p], o_t[p]
                                sc = ps2.tile([128, 2, T], dt.float32, name=f"sc{p}_{cc}", tag="p2")
                                for j in range(nh):
                                    MM(
                                        sc[:, j, 128 * cc :],
                                        krot[64 * j : 64 * j + HD, p, 128 * cc : 128 * cc + 128],
                                        qrot[64 * j : 64 * j + HD, p, 128 * cc :],
                                        start=True,
                                        stop=False,
                                    )
                                for j in range(nh):
                                    MM(
                                        sc[:, j, 128 * cc : 128 * cc + 128],
                                        lt_sb[:],
                                        rt_sb[:],
                                        start=False,
                                        stop=True,
                                        skip_group_check=True,
                                    )
                                ACT(
                                    E_sb[:, 0:nh, 128 * cc :],
                                    sc[:, 0:nh, 128 * cc :],
                                    F.Exp,
                                    scale=SCALE,
                                )
                                for j in range(nh):
                                    h_ = 2 * p + j
                                    MM(
                                        o_p[64 * j : 64 * j + 64, 128 * cc :],
                                        v_sb[:, cc, 64 * h_ : 64 * h_ + 64],
                                        E_sb[:, j, 128 * cc :],
                                        start=(cc == 0),
                                        stop=(cc == NTC - 1 and j == nh - 1),
                                        skip_group_check=True,
                                    )
                        # evacuate the pair accumulators (including the
                        # denominator rows) to SBUF right away — the PSUM bank
                        # frees without waiting on the denominator chain; the
                        # den rows also DMA into the packed den8 tile
                        for p in pg:
                            nh = 1 if p == 3 else 2
                            orw = b1pool.tile([128, T], dt.bfloat16,
                                              name=f"oraw{p}", tag=f"oraw{p}")
                            nc.vector.tensor_copy(orw[0 : 64 * nh, :], o_t[p][0 : 64 * nh, :])
                            for j in range(nh):
                                nc.sync.dma_start(
                                    den8[2 * p + j : 2 * p + j + 1, :],
                                    orw[64 * j : 64 * j + 1, :],
                                )
                            o_raw[p] = orw

                    pend = (s, o_raw, o_sb)

                # ======== MLP (per seq), W2 lagged one fc so the PE never
                # stalls on the silu->mult chain: w2(fc-1) is emitted after
                # gate/up(fc)'s matmuls, by which time gu(fc-1) is ready.
                def mlp_seq(s):
                    h2s = h2[s]
                    xacc = [po.tile([128, T], dt.float32, name=f"mxac{mc}", tag="po")
                            for mc in range(NDC)]

                    def w2_mms(pfc, pgu):
                        for mc in range(NDC):
                            MM(
                                xacc[mc][:],
                                wt["w2"][pfc][:, 128 * mc : 128 * mc + 128],
                                pgu[:],
                                start=(pfc == 0),
                                stop=(pfc == NFC - 1),
                                skip_group_check=True,
                            )

                    prevs = []
                    fc = 0
                    while fc < NFC:
                        nf = 2 if fc + 1 < NFC else 1  # fc pair (or final single)
                        gus = []
                        for i in range(nf):
                            gu_ps = ps2.tile([128, 2, T], dt.float32, name="gu_ps", tag="p2")
                            for gi, wname in ((0, "w1"), (1, "w3")):
                                for kc in range(2):
                                    MM(
                                        gu_ps[:, gi],
                                        wt[wname][kc][:, 128 * (fc + i) : 128 * (fc + i) + 128],
                                        h2s[:, kc],
                                        start=(kc == 0),
                                        stop=False,
                                    )
                            gus.append(gu_ps)
                        # 3rd-chunk gate/up matmuls for the whole fc pair as
                        # up to four concurrent 32-row PE tiles
                        for i in range(nf):
                            fcw = 128 * (fc + i)
                            for gi in range(2):
                                ro = 64 * i + 32 * gi
                                MM(
                                    gus[i][:, gi],
                                    wt["w13c2"][ro : ro + 24, fcw : fcw + 128],
                                    h2s[ro : ro + 24, 2],
                                    start=False,
                                    stop=True,
                                    skip_group_check=True,
                                    tile_position=(ro, 0),
                                )
                        for pfc, pgu in prevs:
                            w2_mms(pfc, pgu)
                        prevs = []
                        for i in range(nf):
                            gate = apool.tile([128, T], dt.bfloat16, name="gate", tag="gate")
                            si = ACT(gate[:], gus[i][:, 0], F.Silu)
                            if mlp_first[0] and fc == 0 and i == 0:
                                _dep_after(si, actdep["last_lnexp"])
                                mlp_first[0] = False
                            actdep["last_silu"] = si
                            gu = apool.tile([128, T], dt.bfloat16, name="gu", tag="gu")
                            TT(gu[:], gus[i][:, 1], gate[:], OP.mult)
                            prevs.append((fc + i, gu))
                        fc += nf
                    for pfc, pgu in prevs:
                        w2_mms(pfc, pgu)
                    for mc in range(NDC):
                        TT(xs[s][:, mc], xacc[mc][:], xs[s][:, mc], OP.add)

                # the whole s3 tail rides one lnexp table window after the
                # first two MLP seqs: the silu table load starts the moment
                # pairs end (no first-w2 stall), the tail's matmuls queue
                # behind ~34us of gate/up work, and BOTH next-layer norm1(s0)
                # and norm1(s1) share the window so the next attention phase
                # opens with two sequences' h1 precomputed.
                mlp_first = [True]
                mlp_seq(1)
                mlp_seq(0)
                attn_tail(*pend)
                h1cache[0] = norm_h(xs[0], f"n1_0_{l + 1}p", "h1_0")
                h1cache[1] = norm_h(xs[1], f"n1_1_{l + 1}p", "h1_1")
                mlp_seq(2)
                mlp_seq(3)

            # ---- final norm (embT has norm_w folded) + logits, with logits
            # lagged one seq so each norm chain hides under the previous
            # seq's logit matmuls
            def logits_seq(s, hf):
                for tc_ in range(NTC):
                    lp = pms.tile([128, V], dt.float32, name="lp", tag="ms")
                    for kc in range(NDC):
                        MM(
                            lp[:],
                            hf[:, kc, 128 * tc_ : 128 * tc_ + 128],
                            embT_t[kc][:],
                            start=(kc == 0),
                            stop=(kc == NDC - 1),
                        )
                    lsb = spool.tile([128, V], dt.float32, name="lsb", tag="lsb")
                    ACT(lsb[:], lp[:], F.Copy)
                    nc.sync.dma_start(logits.ap()[s, tc_], lsb[:])

            hfs = {}
            first_norm = True
            for s in range(n_seqs):
                hfs[s] = h1cache.pop(s, None)
                if hfs[s] is None:
                    hfs[s] = norm_h(xs[s], f"n1f_{s}", f"h1_{s}", after_silu=first_norm)
                    first_norm = False
                if s > 0:
                    logits_seq(s - 1, hfs[s - 1])
            logits_seq(n_seqs - 1, hfs[n_seqs - 1])

    return nc


def _patch_tail_drain(tile_mod):
    """walrus here rejects CTRL instructions with >1 sync wait; split the
    TileContext tail-drain waits across extra SP NOPs (1 wait each)."""
    import concourse.mybir as mybir

    if getattr(tile_mod.TileContext, "_tail_drain_patched", False):
        return

    def _patched(self, tick_clock, wait_clock):
        nc = self.nc
        # This walrus build only accepts one sync wait per instruction:
        # hoist extra waits onto same-engine NOPs inserted just before.
        nsplit = [0]
        for fn in nc.m.functions:
            for bb in fn.blocks:
                insts = bb.instructions
                out = []
                for inst in insts:
                    si = inst.sync_info
                    if si is not None and si.on_wait and len(si.on_wait) > 1:
                        waits = list(si.on_wait)
                        si.on_wait.clear()
                        si.on_wait.append(waits[-1])
                        for w in waits[:-1]:
                            nsplit[0] += 1
                            nop = mybir.InstNoOp(
                                name=f"wsplit-{nsplit[0]}",
                                engine=inst.engine,
                                ins=[],
                                outs=[],
                                sync_info=mybir.SyncInfo(on_wait=[w], on_update=[]),
                                text_hint="wait_split",
                            )
                            out.append(nop)
                    out.append(inst)
                if len(out) != len(insts):
                    insts[:] = out
        drain_inst = nc.sync.drain()
        wait_clock.add_sem_waits(
            drain_inst.ins, tile_mod.ScopedClock({None: tick_clock.global_clock})
        )
        si = drain_inst.ins.sync_info
        waits = list(si.on_wait or [])
        if len(waits) > 1:
            si.on_wait.clear()
            si.on_wait.extend(waits[:1])
            rest = waits[1:]
            for i, w in enumerate(rest):
                nop = nc.sync.nop(nofuse=True, hint=f"tail_wait_split_{i}")
                nsi = nop.ins.sync_info
                if nsi is None:
                    nsi = mybir.SyncInfo(on_wait=[], on_update=[])
                    nop.ins.sync_info = nsi
                nsi.on_wait.append(w)
        nc.all_engine_barrier()
        assert self.sems is not None
        popped = nc._tile_sem_poison_stack.pop()
        assert popped is self._sem_poison
        nc.clear_and_free_semaphores(list(self.sems.allocated().values()))
        nc.all_engine_barrier()

    tile_mod.TileContext._drain_and_barrier = _patched
    tile_mod.TileContext._tail_drain_patched = True


def _in_maps(inputs, n_seqs=NSEQ):
    import ml_dtypes  # noqa: F401

    if "weights" not in _CACHE:
        _CACHE["weights"] = _prep_weights(inputs)
    c = _CACHE["weights"]
    idx = np.asarray(inputs["idx"])
    maps = []
    for core in range(NCORES):
        m = dict(c)
        m["oh"] = _prep_onehot(idx[core * NSEQ : core * NSEQ + n_seqs])
        maps.append(m)
    return maps


def _get_runner():
    """Compile the SPMD executable once; return fn(in_maps) -> logits array
    [NCORES, NSEQ, NTC, 128, V]. Mirrors bass2jax.run_bass_via_pjrt's
    multi-core path but keeps the jitted callable for repeated (timed) runs."""
    if "runner" in _CACHE:
        return _CACHE["runner"]
    import jax
    import concourse.mybir as mybir
    from concourse import bass2jax
    from jax.sharding import Mesh, PartitionSpec
    from jax.experimental.shard_map import shard_map

    bass2jax.install_neuronx_cc_hook()
    if "nc" not in _CACHE:
        _CACHE["nc"] = _build()
    nc = _CACHE["nc"]

    in_names, out_names, out_avals, zero_outs = [], [], [], []
    for alloc in nc.m.functions[0].allocations:
        if not isinstance(alloc, mybir.MemoryLocationSet):
            continue
        name = alloc.memorylocations[0].name
        if alloc.kind == "ExternalInput":
            if not (nc.partition_id_tensor and name == nc.partition_id_tensor.name):
                in_names.append(name)
        elif alloc.kind == "ExternalOutput":
            out_names.append(name)
            shape = tuple(alloc.tensor_shape)
            dtype = mybir.dt.np(alloc.dtype)
            out_avals.append(jax.core.ShapedArray(shape, dtype))
            zero_outs.append(np.zeros(shape, dtype))
    n_params = len(in_names)
    all_names = list(in_names) + list(out_names)
    if nc.partition_id_tensor is not None:
        all_names.append(nc.partition_id_tensor.name)
    donate = tuple(range(n_params, n_params + len(out_names)))

    def _body(*args):
        operands = list(args)
        if nc.partition_id_tensor is not None:
            operands.append(bass2jax.partition_id_tensor())
        outs = bass2jax._bass_exec_p.bind(
            *operands,
            out_avals=tuple(out_avals),
            in_names=tuple(all_names),
            out_names=tuple(out_names),
            lowering_input_output_aliases=(),
            sim_require_finite=True,
            sim_require_nnan=True,
            nc=nc,
        )
        return tuple(outs)

    devices = jax.devices()[:NCORES]
    mesh = Mesh(np.asarray(devices), ("core",))
    in_specs = (PartitionSpec("core"),) * (n_params + len(out_names))
    out_specs = (PartitionSpec("core"),) * len(out_names)
    sharded = jax.jit(
        shard_map(_body, mesh=mesh, in_specs=in_specs, out_specs=out_specs, check_rep=False),
        donate_argnums=donate,
        keep_unused=True,
    )
    sharded_nodonate = jax.jit(
        shard_map(_body, mesh=mesh, in_specs=in_specs, out_specs=out_specs, check_rep=False),
        keep_unused=True,
    )
    oi = out_names.index("logits")
    oshape = out_avals[oi].shape

    def run(maps):
        concat_in = [
            np.concatenate([np.asarray(maps[c][n]) for c in range(NCORES)], axis=0)
            for n in in_names
        ]
        concat_zeros = [
            np.zeros((NCORES * z.shape[0], *z.shape[1:]), z.dtype) for z in zero_outs
        ]
        out_arrs = sharded(*concat_in, *concat_zeros)
        return np.asarray(out_arrs[oi]).reshape(NCORES, *oshape)

    _CACHE["runner"] = run
    _CACHE["runner_parts"] = dict(
        sharded=sharded, sharded_nodonate=sharded_nodonate,
        in_names=in_names, zero_outs=zero_outs, mesh=mesh, oi=oi
    )
    return run


def kernel(**inputs) -> np.ndarray:
    run = _get_runner()
    maps = _in_maps(inputs)
    lg = run(maps)  # [NCORES, NSEQ, NTC, 128, V]
    return lg.reshape(B, T, V)

